# revision 16
# baseline (speedup 1.0000x reference)
"""Bass/Tile kernel for nn_MAlphaAttention (sparse graph attention).

Sharding: 8 cores = 4 batches x 2 head-groups (6 heads each).

Key structure (per core, all matmuls bf16 -> fp32 PSUM):
  P1  qkv^T projection: qkv[n,c'] = sum_c xT[c,n]^T W[c,c'];
      relu drains on ACT (q,k), copy drain (v).
  P2  graph mix fused with transpose: qT[d,m] = sum_n relu_q[n,d] G[n,m],
      G = I + 0.1*mask. Banded: G[n,m] == 0 for |n-m| > 165 (32x32 grid,
      Manhattan radius 5), so out-of-band n-chunks are skipped.
  P3  per head pair (2g, 2g+1): S^T[m,n] = k~T^T q~T (K=64).  The two
      heads of a pair live at PE row groups 0-1 / 2-3 (base partitions
      0/64), so their matmuls are issued back-to-back and run
      concurrently in the PE array (row tiling).  S^T / mask-mult / O^T
      all restricted to the mask band.  st drains PSUM->SBUF bf16 on
      ACT, then the mask multiply runs on DVE at 2x (all-bf16 SBUF).
      z = 1/(s+eps) via ones-column of vplus + DVE reciprocal.
  P4  y[n,e] = sum_hd otT[hd,n] Wout[hd,e]; bf16 partial output, host
      sums the two head-group partials per batch and adds b_out.
"""

import numpy as np
from contextlib import ExitStack

import concourse.bass as bass
from concourse import bacc
import concourse.tile as tile
import concourse.mybir as mybir
from concourse.bass_utils import run_bass_kernel_spmd

F32 = mybir.dt.float32
BF16 = mybir.dt.bfloat16
FP8 = mybir.dt.float8e4
PM = mybir.MatmulPerfMode
AF = mybir.ActivationFunctionType
ALU = mybir.AluOpType

N = 1024          # nodes / sequence
C = 768           # model dim
CG = 384          # channels per head-group (6 heads x 64)
D = 64            # head dim
HG = 6            # heads per group
VW = D + 1        # v columns + ones column
EPS = 1e-6
NT = N // 128     # 8 partition chunks of the node axis
KT = C // 128     # 6 contraction chunks for qkv

# mask[n, m] == 0 for |n - m| > BAND (row-major 32x32 grid, Manhattan
# radius 5 -> linear offset at most 5*32 + 5 = 165).
BAND = 165


def _halves(mc):
    """n-halves h2 whose band overlaps m-chunk mc."""
    out = []
    for h2 in (0, 1):
        if 512 * h2 - BAND <= 128 * mc + 127 and 128 * mc <= 512 * h2 + 511 + BAND:
            out.append(h2)
    return out


def _nchunks(mh):
    """n-chunks j whose band overlaps m-half mh (phase 2)."""
    return [j for j in range(NT)
            if 512 * mh - BAND <= 128 * j + 127
            and 128 * j <= 512 * mh + 511 + BAND]


def _contrib(h2):
    """m-chunks contributing to n-half h2 (phase 3 O^T)."""
    return [mc for mc in range(NT) if h2 in _halves(mc)]


def _w0(mc):
    """Start of the 512-wide n-window covering the band of m-chunk mc:
    support is [128*mc - 165, 128*mc + 292], width 458 <= 512."""
    return max(0, min(128 * mc - 192, 512))


def build_nc():
    nc = bacc.Bacc("TRN2", target_bir_lowering=False, debug=False)

    xT_d = nc.dram_tensor("xt", [C, N], BF16, kind="ExternalInput")
    w_d = nc.dram_tensor("wqkv", [C, 3 * CG], BF16, kind="ExternalInput")
    g_d = nc.dram_tensor("gmix", [N, N], FP8, kind="ExternalInput")
    mt_d = nc.dram_tensor("maskt", [N, N], BF16, kind="ExternalInput")
    w2_d = nc.dram_tensor("wout", [CG, C], BF16, kind="ExternalInput")
    y_d = nc.dram_tensor("y", [N, C], BF16, kind="ExternalOutput")

    with ExitStack() as ctx:
        tc = ctx.enter_context(tile.TileContext(nc))

        persist = ctx.enter_context(tc.tile_pool(name="persist", bufs=1))
        q_nm = persist.tile([128, NT * CG], FP8)       # 16*relu(q), n-major
        k_nm = persist.tile([128, NT * CG], FP8)
        vplus = persist.tile([128, NT * HG * VW], BF16)  # v | ones, n-major
        qT = persist.tile([128, 3 * N], BF16)          # q~^T d-major
        kT = persist.tile([128, 3 * N], BF16)
        otT = persist.tile([128, 3 * N], BF16)         # z-scaled O^T d-major
        G = persist.tile([128, NT * N], FP8)   # 8 * (I + 0.1*mask)
        maskT = persist.tile([128, NT * N], BF16)
        w2 = persist.tile([128, 3 * C], BF16)

        # G/maskT/w2 go on the sync queue so the gpsimd queue can issue
        # the phase-1 xT/w loads immediately (P1 start is DMA-latency
        # sensitive; G/maskT aren't needed until phase 2/3).
        for j in range(NT):
            nc.sync.dma_start(G[:, j * N:(j + 1) * N], g_d[j * 128:(j + 1) * 128, :])
        for j in range(NT):
            nc.sync.dma_start(maskT[:, j * N:(j + 1) * N],
                              mt_d[j * 128:(j + 1) * 128, :])
        for ds in range(3):
            nc.sync.dma_start(w2[:, ds * C:(ds + 1) * C],
                              w2_d[ds * 128:(ds + 1) * 128, :])
        # ================= Phase 1: qkv projection =================
        with tc.tile_pool(name="p1", bufs=1) as p1, \
             tc.tile_pool(name="ps1", bufs=3, space="PSUM") as ps1:
            xT = p1.tile([128, KT * N], BF16)
            w = p1.tile([128, KT * 3 * CG], BF16)
            # split across gpsimd/scalar queues so two queues drive DMA
            # rings in parallel and P1's first group is fed quickly
            for kc in range(KT):
                eng = nc.gpsimd if kc % 2 == 0 else nc.scalar
                eng.dma_start(xT[:, kc * N:(kc + 1) * N],
                              xT_d[kc * 128:(kc + 1) * 128, :])
                eng.dma_start(w[:, kc * 3 * CG:(kc + 1) * 3 * CG],
                              w_d[kc * 128:(kc + 1) * 128, :])
            for j in range(NT):
                vch = vplus[:, j * HG * VW:(j + 1) * HG * VW].rearrange(
                    "p (h w) -> p h w", w=VW)
                nc.gpsimd.memset(vch[:, :, D:VW], 1.0)

            for j in range(NT):
                for p in range(3):  # q, k, v
                    acc = ps1.tile([128, CG], F32, tag="qkvps")
                    for kc in range(KT):
                        nc.tensor.matmul(
                            acc[:],
                            xT[:, kc * N + j * 128: kc * N + (j + 1) * 128],
                            w[:, kc * 3 * CG + p * CG: kc * 3 * CG + (p + 1) * CG],
                            start=(kc == 0), stop=(kc == KT - 1))
                    if p < 2:
                        dst = (q_nm if p == 0 else k_nm)[:, j * CG:(j + 1) * CG]
                        nc.scalar.activation(dst, acc[:], AF.Relu, scale=16.0)
                    else:
                        vch = vplus[:, j * HG * VW:(j + 1) * HG * VW].rearrange(
                            "p (h w) -> p h w", w=VW)
                        nc.vector.tensor_copy(
                            vch[:, :, 0:D],
                            acc[:].rearrange("p (h w) -> p h w", w=D))

        # ============ Phase 2 + 3, pipelined per head pair ============
        # PSUM: "st" [128,1024] x2 (4 banks, shared with P2 accums) +
        # "ot" [65,512] x4 (4 banks) = 8 banks.
        with tc.tile_pool(name="ps23", bufs=1, space="PSUM") as ps23, \
             tc.tile_pool(name="atp", bufs=1) as at_pool, \
             tc.tile_pool(name="sbp", bufs=1) as sb_pool:

            def p2_group(g, src, dstT):
                src3 = src.rearrange("p (j c) -> p j c", c=CG)
                G3 = G.rearrange("p (j m) -> p j m", m=N)
                for mh in (0, 1):
                    acc2 = ps23.tile([128, 512], F32, tag="st", bufs=4,
                                     name=f"g2acc_{g}_{mh}")
                    ds_ = sorted({jj // 2 for jj in _nchunks(mh)})
                    for i, d in enumerate(ds_):
                        nc.tensor.matmul(
                            acc2[:],
                            src3[:, 2 * d:2 * d + 2, g * 128:g * 128 + 128],
                            G3[:, 2 * d:2 * d + 2, mh * 512:mh * 512 + 512],
                            start=(i == 0), stop=(i == len(ds_) - 1),
                            perf_mode=PM.DoubleRow)
                    # q/k carry 16x, G carries 8x -> undo 128x at drain
                    nc.scalar.activation(
                        dstT[:, g * N + mh * 512: g * N + mh * 512 + 512],
                        acc2[:], AF.Copy, scale=1.0 / 128.0)

            at = [{}, {}]

            def s_block(g):
                # one floating 512-wide window per m-chunk: S^T, drain,
                # mask-mult all [128, 512].  Drains split ACT/DVE; mults
                # split DVE/GpSimd to balance the three engines.
                for mc in range(NT):
                    o0 = _w0(mc)
                    sts = [ps23.tile([128, 512], F32, tag="st", bufs=4,
                                     name=f"st{hh}_{mc}")
                           for hh in (0, 1)]
                    for hh in (0, 1):
                        r0 = hh * 64
                        nc.tensor.matmul(
                            sts[hh][:],
                            kT[r0:r0 + 64, g * N + mc * 128: g * N + (mc + 1) * 128],
                            qT[r0:r0 + 64, g * N + o0: g * N + o0 + 512],
                            start=True, stop=True)
                    for hh in (0, 1):
                        stsb = sb_pool.tile([128, 512], BF16, tag="stsb",
                                            bufs=6, name=f"stsb{hh}_{mc}")
                        if (mc + hh) % 4 != 0:
                            nc.scalar.activation(stsb[:], sts[hh][:], AF.Copy)
                        else:
                            nc.vector.tensor_copy(stsb[:], sts[hh][:])
                        a = at_pool.tile([128, 512], BF16,
                                         tag=f"at{hh}_{mc}", bufs=2,
                                         name=f"at{hh}_{mc}")
                        mul_eng = nc.gpsimd if (mc + hh) % 4 == 2 else nc.vector
                        mul_eng.tensor_tensor(
                            a[:], stsb[:], maskT[:, mc * N + o0: mc * N + o0 + 512],
                            op=ALU.mult)
                        at[hh][mc] = a

            def o_head(g, hh):
                h = 2 * g + hh
                r0 = hh * 64
                ots = {}
                for h2 in (0, 1):
                    ot = ps23.tile([VW, 512], F32, tag="ot", bufs=4,
                                   name=f"ot{hh}_{h2}")
                    mcs = _contrib(h2)
                    for i, mc in enumerate(mcs):
                        a = at[hh][mc]
                        o0 = _w0(mc)
                        lo = max(512 * h2, o0)
                        hi = min(512 * h2 + 512, o0 + 512)
                        nc.tensor.matmul(
                            ot[:, lo - 512 * h2: hi - 512 * h2],
                            vplus[:, mc * HG * VW + h * VW: mc * HG * VW + (h + 1) * VW],
                            a[:, lo - o0: hi - o0],
                            start=(i == 0), stop=(i == len(mcs) - 1))
                    ots[h2] = ot
                zrow = sb_pool.tile([1, N], F32, tag="zrow", bufs=2,
                                    name=f"zrow{hh}")
                for h2 in (0, 1):
                    nc.scalar.activation(zrow[:, h2 * 512:(h2 + 1) * 512],
                                         ots[h2][D:VW, :], AF.Copy, bias=EPS)
                zrec = sb_pool.tile([1, N], F32, tag="zrec", bufs=2,
                                    name=f"zrec{hh}")
                nc.vector.reciprocal_approx_fast(zrec[:], zrow[:])
                zb = sb_pool.tile([64, N], F32, tag="zb", bufs=2,
                                  name=f"zb{hh}")
                nc.gpsimd.partition_broadcast(zb[:], zrec[:])

                def zmults(hh=hh, r0=r0, g=g, ots=ots, zb=zb):
                    # deferred: DVE FIFO would otherwise delay the next
                    # pair's mask-mults behind these at the pair boundary
                    for h2 in (0, 1):
                        nc.vector.tensor_tensor(
                            otT[r0:r0 + 64, g * N + h2 * 512: g * N + (h2 + 1) * 512],
                            ots[h2][0:D, :], zb[:, h2 * 512:(h2 + 1) * 512],
                            op=ALU.mult)
                return zmults

            p2_group(0, q_nm, qT)
            p2_group(0, k_nm, kT)
            deferred = []
            for g in range(3):
                s_block(g)
                for fn in deferred:
                    fn()
                deferred = []
                deferred.append(o_head(g, 0))
                if g + 1 < 3:
                    p2_group(g + 1, q_nm, qT)
                deferred.append(o_head(g, 1))
                if g + 1 < 3:
                    p2_group(g + 1, k_nm, kT)
            for fn in deferred:
                fn()

        # ================= Phase 4: output projection ======================
        with tc.tile_pool(name="ps4", bufs=2, space="PSUM") as ps4, \
             tc.tile_pool(name="p4sb", bufs=3) as p4sb:
            for j in range(NT):
                yp = ps4.tile([128, C], F32, tag="yps")
                for ds in range(3):
                    for e0, ew in ((0, 512), (512, 256)):
                        nc.tensor.matmul(
                            yp[:, e0:e0 + ew],
                            otT[:, ds * N + j * 128: ds * N + (j + 1) * 128],
                            w2[:, ds * C + e0: ds * C + e0 + ew],
                            start=(ds == 0), stop=(ds == 2))
                ysb = p4sb.tile([128, C], BF16, tag="ysb")
                nc.vector.tensor_copy(ysb[:], yp[:])
                nc.sync.dma_start(y_d[j * 128:(j + 1) * 128, :], ysb[:])

    nc.compile()
    return nc


_NC_CACHE = {}


def _get_nc():
    if "nc" not in _NC_CACHE:
        _NC_CACHE["nc"] = build_nc()
    return _NC_CACHE["nc"]


def make_in_maps(x, W_qkv, W_out, mask):
    import ml_dtypes
    bf = ml_dtypes.bfloat16
    fp8 = ml_dtypes.float8_e4m3
    G = (8.0 * (np.eye(N, dtype=np.float32) + 0.1 * mask)).astype(fp8)
    maskT = np.ascontiguousarray(mask.T).astype(bf)
    in_maps = []
    for c in range(8):
        b, g = divmod(c, 2)
        xTb = np.ascontiguousarray(x[b].T)
        wq = W_qkv[:, g * CG:(g + 1) * CG]
        wk = W_qkv[:, C + g * CG: C + (g + 1) * CG]
        wv = W_qkv[:, 2 * C + g * CG: 2 * C + (g + 1) * CG]
        w = np.ascontiguousarray(np.concatenate([wq, wk, wv], axis=1))
        w2 = np.ascontiguousarray(W_out[g * CG:(g + 1) * CG, :])
        in_maps.append({"xt": xTb.astype(bf), "wqkv": w.astype(bf),
                        "gmix": G, "maskt": maskT, "wout": w2.astype(bf)})
    return in_maps


def kernel(x, W_qkv, W_out, b_out, mask, _trace=False):
    x = np.asarray(x, dtype=np.float32)
    W_qkv = np.asarray(W_qkv, dtype=np.float32)
    W_out = np.asarray(W_out, dtype=np.float32)
    b_out = np.asarray(b_out, dtype=np.float32)
    mask = np.asarray(mask, dtype=np.float32)

    nc = _get_nc()
    in_maps = make_in_maps(x, W_qkv, W_out, mask)
    res = run_bass_kernel_spmd(nc, in_maps, core_ids=list(range(8)),
                               trace=_trace)
    parts = [r["y"] for r in res.results]
    out = np.empty((4, N, C), dtype=np.float32)
    for b in range(4):
        out[b] = (parts[2 * b].astype(np.float32)
                  + parts[2 * b + 1].astype(np.float32) + b_out)
    if _trace:
        kernel._last_results = res
    return out


# revision 17
# speedup vs baseline: 1.2696x; 1.2696x over previous
"""Bass/Tile kernel for nn_MAlphaAttention (sparse graph attention).

Sharding: 8 cores = 4 batches x 2 head-groups (6 heads each).

Key structure (per core, all matmuls bf16 -> fp32 PSUM):
  P1  qkv^T projection: qkv[n,c'] = sum_c xT[c,n]^T W[c,c'];
      relu drains on ACT (q,k), copy drain (v).
  P2  graph mix fused with transpose: qT[d,m] = sum_n relu_q[n,d] G[n,m],
      G = I + 0.1*mask. Banded: G[n,m] == 0 for |n-m| > 165 (32x32 grid,
      Manhattan radius 5), so out-of-band n-chunks are skipped.
  P3  per head pair (2g, 2g+1): S^T[m,n] = k~T^T q~T (K=64).  The two
      heads of a pair live at PE row groups 0-1 / 2-3 (base partitions
      0/64), so their matmuls are issued back-to-back and run
      concurrently in the PE array (row tiling).  S^T / mask-mult / O^T
      all restricted to the mask band.  st drains PSUM->SBUF bf16 on
      ACT, then the mask multiply runs on DVE at 2x (all-bf16 SBUF).
      z = 1/(s+eps) via ones-column of vplus + DVE reciprocal.
  P4  y[n,e] = sum_hd otT[hd,n] Wout[hd,e]; bf16 partial output, host
      sums the two head-group partials per batch and adds b_out.
"""

import numpy as np
from contextlib import ExitStack

import concourse.bass as bass
from concourse import bacc
import concourse.tile as tile
import concourse.mybir as mybir
from concourse.bass_utils import run_bass_kernel_spmd

F32 = mybir.dt.float32
BF16 = mybir.dt.bfloat16
FP8 = mybir.dt.float8e4
PM = mybir.MatmulPerfMode
AF = mybir.ActivationFunctionType
ALU = mybir.AluOpType

N = 1024          # nodes / sequence
C = 768           # model dim
CG = 384          # channels per head-group (6 heads x 64)
D = 64            # head dim
HG = 6            # heads per group
VW = D + 1        # v columns + ones column
EPS = 1e-6
NT = N // 128     # 8 partition chunks of the node axis
KT = C // 128     # 6 contraction chunks for qkv

# mask[n, m] == 0 for |n - m| > BAND (row-major 32x32 grid, Manhattan
# radius 5 -> linear offset at most 5*32 + 5 = 165).
BAND = 165


def _halves(mc):
    """n-halves h2 whose band overlaps m-chunk mc."""
    out = []
    for h2 in (0, 1):
        if 512 * h2 - BAND <= 128 * mc + 127 and 128 * mc <= 512 * h2 + 511 + BAND:
            out.append(h2)
    return out


def _nchunks(mh):
    """n-chunks j whose band overlaps m-half mh (phase 2)."""
    return [j for j in range(NT)
            if 512 * mh - BAND <= 128 * j + 127
            and 128 * j <= 512 * mh + 511 + BAND]


def _contrib(h2):
    """m-chunks contributing to n-half h2 (phase 3 O^T)."""
    return [mc for mc in range(NT) if h2 in _halves(mc)]


def _w0(mc):
    """Start of the 512-wide n-window covering the band of m-chunk mc:
    support is [128*mc - 165, 128*mc + 292], width 458 <= 512."""
    return max(0, min(128 * mc - 192, 512))


def build_nc():
    nc = bacc.Bacc("TRN2", target_bir_lowering=False, debug=False)

    xT_d = nc.dram_tensor("xt", [C, N], BF16, kind="ExternalInput")
    w_d = nc.dram_tensor("wqkv", [C, 3 * CG], BF16, kind="ExternalInput")
    g_d = nc.dram_tensor("gmix", [N, N], FP8, kind="ExternalInput")
    mt_d = nc.dram_tensor("maskt", [N, N], BF16, kind="ExternalInput")
    w2_d = nc.dram_tensor("wout", [CG, C], BF16, kind="ExternalInput")
    y_d = nc.dram_tensor("y", [N, C], BF16, kind="ExternalOutput")

    with ExitStack() as ctx:
        tc = ctx.enter_context(tile.TileContext(nc))

        persist = ctx.enter_context(tc.tile_pool(name="persist", bufs=1))
        q_nm = persist.tile([128, NT * CG], FP8)       # 16*relu(q), n-major
        k_nm = persist.tile([128, NT * CG], FP8)
        vplus = persist.tile([128, NT * HG * VW], BF16)  # v | ones, n-major
        qT = persist.tile([128, 3 * N], BF16)          # q~^T d-major
        kT = persist.tile([128, 3 * N], BF16)
        otT = persist.tile([128, 3 * N], BF16)         # z-scaled O^T d-major
        G = persist.tile([128, NT * N], FP8)   # 8 * (I + 0.1*mask)
        maskT = persist.tile([128, NT * N], BF16)
        w2 = persist.tile([128, 3 * C], BF16)

        # G/maskT/w2 go on the sync queue so the gpsimd queue can issue
        # the phase-1 xT/w loads immediately (P1 start is DMA-latency
        # sensitive; G/maskT aren't needed until phase 2/3).
        for j in range(NT):
            nc.sync.dma_start(G[:, j * N:(j + 1) * N], g_d[j * 128:(j + 1) * 128, :])
        for j in range(NT):
            nc.sync.dma_start(maskT[:, j * N:(j + 1) * N],
                              mt_d[j * 128:(j + 1) * 128, :])
        for ds in range(3):
            nc.sync.dma_start(w2[:, ds * C:(ds + 1) * C],
                              w2_d[ds * 128:(ds + 1) * 128, :])
        # ================= Phase 1: qkv projection =================
        with tc.tile_pool(name="p1", bufs=1) as p1, \
             tc.tile_pool(name="ps1", bufs=6, space="PSUM") as ps1:
            xT = p1.tile([128, KT * N], BF16)
            w = p1.tile([128, KT * 3 * CG], BF16)
            # split across gpsimd/scalar queues so two queues drive DMA
            # rings in parallel and P1's first group is fed quickly
            for kc in range(KT):
                eng = nc.gpsimd if kc % 2 == 0 else nc.scalar
                eng.dma_start(xT[:, kc * N:(kc + 1) * N],
                              xT_d[kc * 128:(kc + 1) * 128, :])
                eng.dma_start(w[:, kc * 3 * CG:(kc + 1) * 3 * CG],
                              w_d[kc * 128:(kc + 1) * 128, :])
            for j in range(NT):
                vch = vplus[:, j * HG * VW:(j + 1) * HG * VW].rearrange(
                    "p (h w) -> p h w", w=VW)
                nc.gpsimd.memset(vch[:, :, D:VW], 1.0)

            for j in range(NT):
                for p in range(3):  # q, k, v
                    acc = ps1.tile([128, CG], F32, tag="qkvps")
                    for kc in range(KT):
                        nc.tensor.matmul(
                            acc[:],
                            xT[:, kc * N + j * 128: kc * N + (j + 1) * 128],
                            w[:, kc * 3 * CG + p * CG: kc * 3 * CG + (p + 1) * CG],
                            start=(kc == 0), stop=(kc == KT - 1))
                    if p < 2:
                        dst = (q_nm if p == 0 else k_nm)[:, j * CG:(j + 1) * CG]
                        nc.scalar.activation(dst, acc[:], AF.Relu, scale=16.0)
                    else:
                        vch = vplus[:, j * HG * VW:(j + 1) * HG * VW].rearrange(
                            "p (h w) -> p h w", w=VW)
                        nc.vector.tensor_copy(
                            vch[:, :, 0:D],
                            acc[:].rearrange("p (h w) -> p h w", w=D))

        # ============ Phase 2 + 3, pipelined per head pair ============
        # PSUM: "st" [128,1024] x2 (4 banks, shared with P2 accums) +
        # "ot" [65,512] x4 (4 banks) = 8 banks.
        with tc.tile_pool(name="ps23", bufs=1, space="PSUM") as ps23, \
             tc.tile_pool(name="atp", bufs=1) as at_pool, \
             tc.tile_pool(name="sbp", bufs=1) as sb_pool:

            def p2_group(g, src, dstT):
                src3 = src.rearrange("p (j c) -> p j c", c=CG)
                G3 = G.rearrange("p (j m) -> p j m", m=N)
                for mh in (0, 1):
                    acc2 = ps23.tile([128, 512], F32, tag="st", bufs=4,
                                     name=f"g2acc_{g}_{mh}")
                    ds_ = sorted({jj // 2 for jj in _nchunks(mh)})
                    for i, d in enumerate(ds_):
                        nc.tensor.matmul(
                            acc2[:],
                            src3[:, 2 * d:2 * d + 2, g * 128:g * 128 + 128],
                            G3[:, 2 * d:2 * d + 2, mh * 512:mh * 512 + 512],
                            start=(i == 0), stop=(i == len(ds_) - 1),
                            perf_mode=PM.DoubleRow)
                    # q/k carry 16x, G carries 8x -> undo 128x at drain
                    nc.scalar.activation(
                        dstT[:, g * N + mh * 512: g * N + mh * 512 + 512],
                        acc2[:], AF.Copy, scale=1.0 / 128.0)

            at = [{}, {}]

            def s_block(g):
                # one floating 512-wide window per m-chunk: S^T, drain,
                # mask-mult all [128, 512].  Drains split ACT/DVE; mults
                # split DVE/GpSimd to balance the three engines.
                for mc in range(NT):
                    o0 = _w0(mc)
                    sts = [ps23.tile([128, 512], F32, tag="st", bufs=4,
                                     name=f"st{hh}_{mc}")
                           for hh in (0, 1)]
                    for hh in (0, 1):
                        r0 = hh * 64
                        nc.tensor.matmul(
                            sts[hh][:],
                            kT[r0:r0 + 64, g * N + mc * 128: g * N + (mc + 1) * 128],
                            qT[r0:r0 + 64, g * N + o0: g * N + o0 + 512],
                            start=True, stop=True)
                    for hh in (0, 1):
                        stsb = sb_pool.tile([128, 512], BF16, tag="stsb",
                                            bufs=6, name=f"stsb{hh}_{mc}")
                        if (mc + hh) % 4 != 0:
                            nc.scalar.activation(stsb[:], sts[hh][:], AF.Copy)
                        else:
                            nc.vector.tensor_copy(stsb[:], sts[hh][:])
                        a = at_pool.tile([128, 512], BF16,
                                         tag=f"at{hh}_{mc}", bufs=2,
                                         name=f"at{hh}_{mc}")
                        nc.vector.tensor_tensor(
                            a[:], stsb[:], maskT[:, mc * N + o0: mc * N + o0 + 512],
                            op=ALU.mult)
                        at[hh][mc] = a

            def o_head(g, hh):
                h = 2 * g + hh
                r0 = hh * 64
                ots = {}
                for h2 in (0, 1):
                    ot = ps23.tile([VW, 512], F32, tag="ot", bufs=4,
                                   name=f"ot{hh}_{h2}")
                    mcs = _contrib(h2)
                    for i, mc in enumerate(mcs):
                        a = at[hh][mc]
                        o0 = _w0(mc)
                        lo = max(512 * h2, o0)
                        hi = min(512 * h2 + 512, o0 + 512)
                        nc.tensor.matmul(
                            ot[:, lo - 512 * h2: hi - 512 * h2],
                            vplus[:, mc * HG * VW + h * VW: mc * HG * VW + (h + 1) * VW],
                            a[:, lo - o0: hi - o0],
                            start=(i == 0), stop=(i == len(mcs) - 1))
                    ots[h2] = ot
                zrow = sb_pool.tile([1, N], F32, tag="zrow", bufs=2,
                                    name=f"zrow{hh}")
                for h2 in (0, 1):
                    nc.scalar.activation(zrow[:, h2 * 512:(h2 + 1) * 512],
                                         ots[h2][D:VW, :], AF.Copy, bias=EPS)
                zrec = sb_pool.tile([1, N], F32, tag="zrec", bufs=2,
                                    name=f"zrec{hh}")
                nc.vector.reciprocal_approx_fast(zrec[:], zrow[:])
                zb = sb_pool.tile([64, N], F32, tag="zb", bufs=2,
                                  name=f"zb{hh}")
                nc.gpsimd.partition_broadcast(zb[:], zrec[:])

                def zmults(hh=hh, r0=r0, g=g, ots=ots, zb=zb):
                    # deferred: DVE FIFO would otherwise delay the next
                    # pair's mask-mults behind these at the pair boundary
                    for h2 in (0, 1):
                        nc.vector.tensor_tensor(
                            otT[r0:r0 + 64, g * N + h2 * 512: g * N + (h2 + 1) * 512],
                            ots[h2][0:D, :], zb[:, h2 * 512:(h2 + 1) * 512],
                            op=ALU.mult)
                return zmults

            p2_group(0, q_nm, qT)
            p2_group(0, k_nm, kT)
            deferred = []
            for g in range(3):
                s_block(g)
                for fn in deferred:
                    fn()
                deferred = []
                deferred.append(o_head(g, 0))
                if g + 1 < 3:
                    p2_group(g + 1, q_nm, qT)
                deferred.append(o_head(g, 1))
                if g + 1 < 3:
                    p2_group(g + 1, k_nm, kT)
                else:
                    # last pair: run z-scales now; P4's ds=2 waits on them
                    for fn in deferred:
                        fn()
                    deferred = []

        # ================= Phase 4: output projection ======================
        # ds-major within groups of 4 j-chunks: the ds=0/1 matmuls only
        # need pairs 0/1 and overlap the last pair's O/z tail; only the
        # ds=2 matmuls wait on the final zmults.
        with tc.tile_pool(name="ps4", bufs=1, space="PSUM") as ps4, \
             tc.tile_pool(name="p4sb", bufs=3) as p4sb:
            for jg in (0, 1):
                yps = [ps4.tile([128, C], F32, tag=f"yps{jj}", bufs=1,
                                name=f"yp{jg}_{jj}")
                       for jj in range(4)]
                for ds in range(3):
                    for jj in range(4):
                        j = jg * 4 + jj
                        for e0, ew in ((0, 512), (512, 256)):
                            nc.tensor.matmul(
                                yps[jj][:, e0:e0 + ew],
                                otT[:, ds * N + j * 128: ds * N + (j + 1) * 128],
                                w2[:, ds * C + e0: ds * C + e0 + ew],
                                start=(ds == 0), stop=(ds == 2))
                for jj in range(4):
                    j = jg * 4 + jj
                    ysb = p4sb.tile([128, C], BF16, tag="ysb",
                                    name=f"ysb{j}")
                    nc.vector.tensor_copy(ysb[:], yps[jj][:])
                    nc.sync.dma_start(y_d[j * 128:(j + 1) * 128, :], ysb[:])

    nc.compile()
    return nc


_NC_CACHE = {}


def _get_nc():
    if "nc" not in _NC_CACHE:
        _NC_CACHE["nc"] = build_nc()
    return _NC_CACHE["nc"]


def make_in_maps(x, W_qkv, W_out, mask):
    import ml_dtypes
    bf = ml_dtypes.bfloat16
    fp8 = ml_dtypes.float8_e4m3
    G = (8.0 * (np.eye(N, dtype=np.float32) + 0.1 * mask)).astype(fp8)
    maskT = np.ascontiguousarray(mask.T).astype(bf)
    in_maps = []
    for c in range(8):
        b, g = divmod(c, 2)
        xTb = np.ascontiguousarray(x[b].T)
        wq = W_qkv[:, g * CG:(g + 1) * CG]
        wk = W_qkv[:, C + g * CG: C + (g + 1) * CG]
        wv = W_qkv[:, 2 * C + g * CG: 2 * C + (g + 1) * CG]
        w = np.ascontiguousarray(np.concatenate([wq, wk, wv], axis=1))
        w2 = np.ascontiguousarray(W_out[g * CG:(g + 1) * CG, :])
        in_maps.append({"xt": xTb.astype(bf), "wqkv": w.astype(bf),
                        "gmix": G, "maskt": maskT, "wout": w2.astype(bf)})
    return in_maps


def kernel(x, W_qkv, W_out, b_out, mask, _trace=False):
    x = np.asarray(x, dtype=np.float32)
    W_qkv = np.asarray(W_qkv, dtype=np.float32)
    W_out = np.asarray(W_out, dtype=np.float32)
    b_out = np.asarray(b_out, dtype=np.float32)
    mask = np.asarray(mask, dtype=np.float32)

    nc = _get_nc()
    in_maps = make_in_maps(x, W_qkv, W_out, mask)
    res = run_bass_kernel_spmd(nc, in_maps, core_ids=list(range(8)),
                               trace=_trace)
    parts = [r["y"] for r in res.results]
    out = np.empty((4, N, C), dtype=np.float32)
    for b in range(4):
        out[b] = (parts[2 * b].astype(np.float32)
                  + parts[2 * b + 1].astype(np.float32) + b_out)
    if _trace:
        kernel._last_results = res
    return out


# revision 18
# speedup vs baseline: 1.3141x; 1.0350x over previous
"""Bass/Tile kernel for nn_MAlphaAttention (sparse graph attention).

Sharding: 8 cores = 4 batches x 2 head-groups (6 heads each).

Key structure (per core, all matmuls bf16 -> fp32 PSUM):
  P1  qkv^T projection: qkv[n,c'] = sum_c xT[c,n]^T W[c,c'];
      relu drains on ACT (q,k), copy drain (v).
  P2  graph mix fused with transpose: qT[d,m] = sum_n relu_q[n,d] G[n,m],
      G = I + 0.1*mask. Banded: G[n,m] == 0 for |n-m| > 165 (32x32 grid,
      Manhattan radius 5), so out-of-band n-chunks are skipped.
  P3  per head pair (2g, 2g+1): S^T[m,n] = k~T^T q~T (K=64).  The two
      heads of a pair live at PE row groups 0-1 / 2-3 (base partitions
      0/64), so their matmuls are issued back-to-back and run
      concurrently in the PE array (row tiling).  S^T / mask-mult / O^T
      all restricted to the mask band.  st drains PSUM->SBUF bf16 on
      ACT, then the mask multiply runs on DVE at 2x (all-bf16 SBUF).
      z = 1/(s+eps) via ones-column of vplus + DVE reciprocal.
  P4  y[n,e] = sum_hd otT[hd,n] Wout[hd,e]; bf16 partial output, host
      sums the two head-group partials per batch and adds b_out.
"""

import numpy as np
from contextlib import ExitStack

import concourse.bass as bass
from concourse import bacc
import concourse.tile as tile
import concourse.mybir as mybir
from concourse.bass_utils import run_bass_kernel_spmd

F32 = mybir.dt.float32
BF16 = mybir.dt.bfloat16
FP8 = mybir.dt.float8e4
PM = mybir.MatmulPerfMode
AF = mybir.ActivationFunctionType
ALU = mybir.AluOpType

N = 1024          # nodes / sequence
C = 768           # model dim
CG = 384          # channels per head-group (6 heads x 64)
D = 64            # head dim
HG = 6            # heads per group
VW = D + 1        # v columns + ones column
EPS = 1e-6
NT = N // 128     # 8 partition chunks of the node axis
KT = C // 128     # 6 contraction chunks for qkv

# mask[n, m] == 0 for |n - m| > BAND (row-major 32x32 grid, Manhattan
# radius 5 -> linear offset at most 5*32 + 5 = 165).
BAND = 165


def _halves(mc):
    """n-halves h2 whose band overlaps m-chunk mc."""
    out = []
    for h2 in (0, 1):
        if 512 * h2 - BAND <= 128 * mc + 127 and 128 * mc <= 512 * h2 + 511 + BAND:
            out.append(h2)
    return out


def _nchunks(mh):
    """n-chunks j whose band overlaps m-half mh (phase 2)."""
    return [j for j in range(NT)
            if 512 * mh - BAND <= 128 * j + 127
            and 128 * j <= 512 * mh + 511 + BAND]


def _contrib(h2):
    """m-chunks contributing to n-half h2 (phase 3 O^T)."""
    return [mc for mc in range(NT) if h2 in _halves(mc)]


WINW = 448  # exact mask support width per m-chunk (measured)


def _w0(mc):
    """Start of the 448-wide n-window covering the mask support of
    m-chunk mc: support is exactly [128*mc - 160, 128*mc + 287]."""
    return max(0, min(128 * mc - 160, N - WINW))


def build_nc():
    nc = bacc.Bacc("TRN2", target_bir_lowering=False, debug=False)

    xT_d = nc.dram_tensor("xt", [C, N], BF16, kind="ExternalInput")
    w_d = nc.dram_tensor("wqkv", [C, 3 * CG], BF16, kind="ExternalInput")
    g_d = nc.dram_tensor("gmix", [N, N], FP8, kind="ExternalInput")
    mt_d = nc.dram_tensor("maskt", [N, N], BF16, kind="ExternalInput")
    w2_d = nc.dram_tensor("wout", [CG, C], BF16, kind="ExternalInput")
    y_d = nc.dram_tensor("y", [N, C], BF16, kind="ExternalOutput")

    with ExitStack() as ctx:
        tc = ctx.enter_context(tile.TileContext(nc))

        persist = ctx.enter_context(tc.tile_pool(name="persist", bufs=1))
        q_nm = persist.tile([128, NT * CG], FP8)       # 16*relu(q), n-major
        k_nm = persist.tile([128, NT * CG], FP8)
        vplus = persist.tile([128, NT * HG * VW], BF16)  # v | ones, n-major
        qT = persist.tile([128, 3 * N], BF16)          # q~^T d-major
        kT = persist.tile([128, 3 * N], BF16)
        otT = persist.tile([128, 3 * N], BF16)         # z-scaled O^T d-major
        G = persist.tile([128, NT * N], FP8)   # 8 * (I + 0.1*mask)
        maskT = persist.tile([128, NT * N], BF16)
        w2 = persist.tile([128, 3 * C], BF16)

        # ================= Phase 1: qkv projection =================
        with tc.tile_pool(name="p1", bufs=1) as p1, \
             tc.tile_pool(name="ps1", bufs=6, space="PSUM") as ps1:
            xT = p1.tile([128, KT * N], BF16)
            w = p1.tile([128, KT * 3 * CG], BF16)
            # xT/w first, spread over 3 queues, so no later transfer
            # contends for DMA rings before P1's data lands; G/maskT/w2
            # (needed only by phase 2/3) issue after.
            qs = [nc.gpsimd, nc.scalar, nc.sync]
            for kc in range(KT):
                eng = qs[kc % 3]
                eng.dma_start(xT[:, kc * N:(kc + 1) * N],
                              xT_d[kc * 128:(kc + 1) * 128, :])
                eng.dma_start(w[:, kc * 3 * CG:(kc + 1) * 3 * CG],
                              w_d[kc * 128:(kc + 1) * 128, :])
            for j in range(NT):
                qs[j % 3].dma_start(G[:, j * N:(j + 1) * N],
                                    g_d[j * 128:(j + 1) * 128, :])
            for j in range(NT):
                qs[j % 3].dma_start(maskT[:, j * N:(j + 1) * N],
                                    mt_d[j * 128:(j + 1) * 128, :])
            for ds in range(3):
                qs[ds % 3].dma_start(w2[:, ds * C:(ds + 1) * C],
                                     w2_d[ds * 128:(ds + 1) * 128, :])
            for j in range(NT):
                vch = vplus[:, j * HG * VW:(j + 1) * HG * VW].rearrange(
                    "p (h w) -> p h w", w=VW)
                nc.gpsimd.memset(vch[:, :, D:VW], 1.0)

            for j in range(NT):
                for p in range(3):  # q, k, v
                    acc = ps1.tile([128, CG], F32, tag="qkvps")
                    for kc in range(KT):
                        nc.tensor.matmul(
                            acc[:],
                            xT[:, kc * N + j * 128: kc * N + (j + 1) * 128],
                            w[:, kc * 3 * CG + p * CG: kc * 3 * CG + (p + 1) * CG],
                            start=(kc == 0), stop=(kc == KT - 1))
                    if p < 2:
                        dst = (q_nm if p == 0 else k_nm)[:, j * CG:(j + 1) * CG]
                        nc.scalar.activation(dst, acc[:], AF.Relu, scale=16.0)
                    else:
                        vch = vplus[:, j * HG * VW:(j + 1) * HG * VW].rearrange(
                            "p (h w) -> p h w", w=VW)
                        nc.vector.tensor_copy(
                            vch[:, :, 0:D],
                            acc[:].rearrange("p (h w) -> p h w", w=D))

        # ============ Phase 2 + 3, pipelined per head pair ============
        # PSUM: "st" [128,1024] x2 (4 banks, shared with P2 accums) +
        # "ot" [65,512] x4 (4 banks) = 8 banks.
        with tc.tile_pool(name="ps23", bufs=1, space="PSUM") as ps23, \
             tc.tile_pool(name="atp", bufs=1) as at_pool, \
             tc.tile_pool(name="sbp", bufs=1) as sb_pool:

            def p2_group(g, src, dstT):
                src3 = src.rearrange("p (j c) -> p j c", c=CG)
                G3 = G.rearrange("p (j m) -> p j m", m=N)
                for mh in (0, 1):
                    acc2 = ps23.tile([128, 512], F32, tag="st", bufs=4,
                                     name=f"g2acc_{g}_{mh}")
                    ds_ = sorted({jj // 2 for jj in _nchunks(mh)})
                    for i, d in enumerate(ds_):
                        nc.tensor.matmul(
                            acc2[:],
                            src3[:, 2 * d:2 * d + 2, g * 128:g * 128 + 128],
                            G3[:, 2 * d:2 * d + 2, mh * 512:mh * 512 + 512],
                            start=(i == 0), stop=(i == len(ds_) - 1),
                            perf_mode=PM.DoubleRow)
                    # q/k carry 16x, G carries 8x -> undo 128x at drain
                    nc.scalar.activation(
                        dstT[:, g * N + mh * 512: g * N + mh * 512 + 512],
                        acc2[:], AF.Copy, scale=1.0 / 128.0)

            at = [{}, {}]

            def s_block(g):
                # one floating 512-wide window per m-chunk: S^T, drain,
                # mask-mult all [128, 512].  Drains split ACT/DVE; mults
                # split DVE/GpSimd to balance the three engines.
                for mc in range(NT):
                    o0 = _w0(mc)
                    sts = [ps23.tile([128, WINW], F32, tag="st", bufs=4,
                                     name=f"st{hh}_{mc}")
                           for hh in (0, 1)]
                    for hh in (0, 1):
                        r0 = hh * 64
                        nc.tensor.matmul(
                            sts[hh][:],
                            kT[r0:r0 + 64, g * N + mc * 128: g * N + (mc + 1) * 128],
                            qT[r0:r0 + 64, g * N + o0: g * N + o0 + WINW],
                            start=True, stop=True)
                    for hh in (0, 1):
                        stsb = sb_pool.tile([128, WINW], BF16, tag="stsb",
                                            bufs=6, name=f"stsb{hh}_{mc}")
                        if (mc + hh) % 4 != 0:
                            nc.scalar.activation(stsb[:], sts[hh][:], AF.Copy)
                        else:
                            nc.vector.tensor_copy(stsb[:], sts[hh][:])
                        a = at_pool.tile([128, WINW], BF16,
                                         tag=f"at{hh}_{mc}", bufs=2,
                                         name=f"at{hh}_{mc}")
                        nc.vector.tensor_tensor(
                            a[:], stsb[:], maskT[:, mc * N + o0: mc * N + o0 + WINW],
                            op=ALU.mult)
                        at[hh][mc] = a

            def o_head(g, hh):
                h = 2 * g + hh
                r0 = hh * 64
                ots = {}
                for h2 in (0, 1):
                    ot = ps23.tile([VW, 512], F32, tag="ot", bufs=4,
                                   name=f"ot{hh}_{h2}")
                    mcs = _contrib(h2)
                    for i, mc in enumerate(mcs):
                        a = at[hh][mc]
                        o0 = _w0(mc)
                        lo = max(512 * h2, o0)
                        hi = min(512 * h2 + 512, o0 + WINW)
                        nc.tensor.matmul(
                            ot[:, lo - 512 * h2: hi - 512 * h2],
                            vplus[:, mc * HG * VW + h * VW: mc * HG * VW + (h + 1) * VW],
                            a[:, lo - o0: hi - o0],
                            start=(i == 0), stop=(i == len(mcs) - 1))
                    ots[h2] = ot
                zrow = sb_pool.tile([1, N], F32, tag="zrow", bufs=2,
                                    name=f"zrow{hh}")
                for h2 in (0, 1):
                    nc.scalar.activation(zrow[:, h2 * 512:(h2 + 1) * 512],
                                         ots[h2][D:VW, :], AF.Copy, bias=EPS)
                zrec = sb_pool.tile([1, N], F32, tag="zrec", bufs=2,
                                    name=f"zrec{hh}")
                nc.vector.reciprocal_approx_fast(zrec[:], zrow[:])
                zb = sb_pool.tile([64, N], F32, tag="zb", bufs=2,
                                  name=f"zb{hh}")
                nc.gpsimd.partition_broadcast(zb[:], zrec[:])

                def zmults(hh=hh, r0=r0, g=g, ots=ots, zb=zb):
                    # deferred: DVE FIFO would otherwise delay the next
                    # pair's mask-mults behind these at the pair boundary
                    for h2 in (0, 1):
                        nc.vector.tensor_tensor(
                            otT[r0:r0 + 64, g * N + h2 * 512: g * N + (h2 + 1) * 512],
                            ots[h2][0:D, :], zb[:, h2 * 512:(h2 + 1) * 512],
                            op=ALU.mult)
                return zmults

            p2_group(0, q_nm, qT)
            p2_group(0, k_nm, kT)
            deferred = []
            for g in range(3):
                s_block(g)
                for fn in deferred:
                    fn()
                deferred = []
                deferred.append(o_head(g, 0))
                if g + 1 < 3:
                    p2_group(g + 1, q_nm, qT)
                deferred.append(o_head(g, 1))
                if g + 1 < 3:
                    p2_group(g + 1, k_nm, kT)
                else:
                    # last pair: run z-scales now; P4's ds=2 waits on them
                    for fn in deferred:
                        fn()
                    deferred = []

        # ================= Phase 4: output projection ======================
        # ds-major within groups of 4 j-chunks: the ds=0/1 matmuls only
        # need pairs 0/1 and overlap the last pair's O/z tail; only the
        # ds=2 matmuls wait on the final zmults.
        with tc.tile_pool(name="ps4", bufs=1, space="PSUM") as ps4, \
             tc.tile_pool(name="p4sb", bufs=3) as p4sb:
            for jg in (0, 1):
                yps = [ps4.tile([128, C], F32, tag=f"yps{jj}", bufs=1,
                                name=f"yp{jg}_{jj}")
                       for jj in range(4)]
                for ds in range(3):
                    for jj in range(4):
                        j = jg * 4 + jj
                        for e0, ew in ((0, 512), (512, 256)):
                            nc.tensor.matmul(
                                yps[jj][:, e0:e0 + ew],
                                otT[:, ds * N + j * 128: ds * N + (j + 1) * 128],
                                w2[:, ds * C + e0: ds * C + e0 + ew],
                                start=(ds == 0), stop=(ds == 2))
                for jj in range(4):
                    j = jg * 4 + jj
                    ysb = p4sb.tile([128, C], BF16, tag="ysb",
                                    name=f"ysb{j}")
                    nc.vector.tensor_copy(ysb[:], yps[jj][:])
                    yeng = nc.sync if jj % 2 == 0 else nc.gpsimd
                    yeng.dma_start(y_d[j * 128:(j + 1) * 128, :], ysb[:])

    nc.compile()
    return nc


_NC_CACHE = {}


def _get_nc():
    if "nc" not in _NC_CACHE:
        _NC_CACHE["nc"] = build_nc()
    return _NC_CACHE["nc"]


def make_in_maps(x, W_qkv, W_out, mask):
    import ml_dtypes
    bf = ml_dtypes.bfloat16
    fp8 = ml_dtypes.float8_e4m3
    G = (8.0 * (np.eye(N, dtype=np.float32) + 0.1 * mask)).astype(fp8)
    maskT = np.ascontiguousarray(mask.T).astype(bf)
    in_maps = []
    for c in range(8):
        b, g = divmod(c, 2)
        xTb = np.ascontiguousarray(x[b].T)
        wq = W_qkv[:, g * CG:(g + 1) * CG]
        wk = W_qkv[:, C + g * CG: C + (g + 1) * CG]
        wv = W_qkv[:, 2 * C + g * CG: 2 * C + (g + 1) * CG]
        w = np.ascontiguousarray(np.concatenate([wq, wk, wv], axis=1))
        w2 = np.ascontiguousarray(W_out[g * CG:(g + 1) * CG, :])
        in_maps.append({"xt": xTb.astype(bf), "wqkv": w.astype(bf),
                        "gmix": G, "maskt": maskT, "wout": w2.astype(bf)})
    return in_maps


def kernel(x, W_qkv, W_out, b_out, mask, _trace=False):
    x = np.asarray(x, dtype=np.float32)
    W_qkv = np.asarray(W_qkv, dtype=np.float32)
    W_out = np.asarray(W_out, dtype=np.float32)
    b_out = np.asarray(b_out, dtype=np.float32)
    mask = np.asarray(mask, dtype=np.float32)

    nc = _get_nc()
    in_maps = make_in_maps(x, W_qkv, W_out, mask)
    res = run_bass_kernel_spmd(nc, in_maps, core_ids=list(range(8)),
                               trace=_trace)
    parts = [r["y"] for r in res.results]
    out = np.empty((4, N, C), dtype=np.float32)
    for b in range(4):
        out[b] = (parts[2 * b].astype(np.float32)
                  + parts[2 * b + 1].astype(np.float32) + b_out)
    if _trace:
        kernel._last_results = res
    return out
